# revision 14
# baseline (speedup 1.0000x reference)
"""DSAttention (de-stationary causal attention) Trainium2 Bass kernel.

Problem: B=4, L=S=2048, H=8, E=D=64, f32.
  scores = (Q @ K^T) * tau[b] + delta[b, j]
  A = softmax(scale * scores + causal mask), scale = 1/sqrt(E)
  out = A @ V
Sharding: B*H = 32 independent (b,h) attention heads -> 4 per core on 8 cores.

Device algorithm (per (b,h) pair), S^T formulation so the softmax reduction
falls out of the PE:
  - Host pre-transposes Q,K to [E, L] ("e on partitions"), casts to bf16, and
    folds the de-stationary terms into the matmul:
      qk[0:64, i]      = 0.125 * tau[b] * Q[i, :]^T ; qk[64, i] = 1        (qt)
      qk[0:64, 2048+j] = K[j, :]^T ; qk[64, 2048+j] = 0.125*delta[b,j]     (kt)
    => S'^T[j, i] = 0.125 * (tau * (Q K^T) + delta)[j, i]  (the exact logits)
  - bf16 operands run the PE at twice the fp32 streaming rate.
  - Causal mask: multiplicative 0/1 [128,128] bf16 trim on the exp'd
    diagonal block (DVE).  PV issues the trim-independent PSUM bank first
    so the PE never waits on the DVE.
  - One wide exp per chunk (ACT, f32 PSUM -> bf16 SBUF).
  - PV: V is augmented with a ones column (host) so a single accumulating
    matmul yields the numerator O'^T (rows 0..63) AND the softmax denominator
    (row 64) in PSUM.
  - Unnormalized [65, 2048] result returns to HBM; the host divides by row 64
    and transposes into the (B, L, H, D) output.
"""

import numpy as np
import ml_dtypes

B, L, SEQ, H, E, D = 4, 2048, 2048, 8, 64, 64
N_CORES = 8
PAIRS = (B * H) // N_CORES  # 4 (b,h) pairs per core
SCALE = 1.0 / float(np.sqrt(E))  # 0.125
JC = 128               # j-chunk (key rows per tile, PSUM partition dim)
IH = 1024              # i-half width (PSUM free dim budget)
N_CHUNKS = SEQ // JC   # 16

_CACHED = {}


def _build_bass(reps=1):
    key = ("nc", reps)
    if key in _CACHED:
        return _CACHED[key]
    import concourse.mybir as mybir
    import concourse.tile as tile
    from concourse import bacc

    f32 = mybir.dt.float32
    bf16 = mybir.dt.bfloat16
    EXP = mybir.ActivationFunctionType.Exp

    nc = bacc.Bacc("TRN2", target_bir_lowering=False, debug=False)

    qk = nc.dram_tensor("qk", [PAIRS, E + 1, 2 * L], bf16,
                        kind="ExternalInput").ap()
    vp = nc.dram_tensor("vp", [PAIRS, JC, N_CHUNKS * (D + 1)], bf16,
                        kind="ExternalInput").ap()
    trim = nc.dram_tensor("trim", [JC, JC], bf16, kind="ExternalInput").ap()
    o = nc.dram_tensor("o", [PAIRS, D + 1, L], f32, kind="ExternalOutput").ap()

    with tile.TileContext(nc) as tc:
        with (
            tc.tile_pool(name="const", bufs=1) as const_pool,
            tc.tile_pool(name="qkp", bufs=2) as qk_pool,
            tc.tile_pool(name="vpool", bufs=2) as v_pool,
            tc.tile_pool(name="et", bufs=4) as et_pool,
            tc.tile_pool(name="ot", bufs=2) as ot_pool,
            tc.tile_pool(name="ps", bufs=2, space="PSUM") as ps_pool,
            tc.tile_pool(name="po", bufs=2, space="PSUM") as po_pool,
        ):
            trim_t = const_pool.tile([JC, JC], bf16, name="trim_t")
            nc.sync.dma_start(out=trim_t[:], in_=trim[:])

            for rep in range(reps):
              for p in range(PAIRS):
                  # split DMAs: the first QK only waits on kt + qt half 0
                  kt_t = qk_pool.tile([E + 1, L], bf16, tag="kt",
                                      name=f"kt{rep}_{p}")
                  qt_t = [qk_pool.tile([E + 1, IH], bf16, tag=f"qt{h}",
                                       name=f"qt{rep}_{p}_{h}")
                          for h in range(L // IH)]
                  vp_t = v_pool.tile([JC, N_CHUNKS * (D + 1)], bf16, tag="vp",
                                     name=f"vp{rep}_{p}")
                  nc.sync.dma_start(out=kt_t[:], in_=qk[p][:, L:2 * L])
                  nc.sync.dma_start(out=qt_t[0][:], in_=qk[p][:, 0:IH])
                  nc.sync.dma_start(out=vp_t[:], in_=vp[p])
                  nc.sync.dma_start(out=qt_t[1][:], in_=qk[p][:, IH:2 * IH])

                  for half in range(L // IH):
                      i_lo = half * IH
                      po_t = po_pool.tile([D + 1, IH], f32, tag="po",
                                          name=f"po{rep}_{p}_{half}")
                      nchunks = (i_lo + IH) // JC
                      # last chunk touching each 512-col PSUM bank (the sim's
                      # accumulation-group stop flag is bank-granular)
                      last_c = {}
                      for c in range(nchunks):
                          xc = max(0, JC * c - i_lo)
                          for b0 in range(0, IH, 512):
                              if max(xc, b0) < b0 + 512:
                                  last_c[b0] = c
                      for c in range(nchunks - 4):
                          j0 = JC * c
                          a0 = max(i_lo, j0)     # first valid (causal) i col
                          x = a0 - i_lo          # offset within the i-half
                          ps_t = ps_pool.tile([JC, IH], f32, tag="ps",
                                              name=f"ps{rep}_{p}_{half}_{c}")
                          # S'^T = kt_chunk.T @ qt into per-bank slices
                          for b0 in (0, 512):
                              lo = max(x, b0)
                              b1 = b0 + 512
                              if lo < b1:
                                  nc.tensor.matmul(
                                      ps_t[:, lo:b1],
                                      kt_t[:, j0:j0 + JC],
                                      qt_t[half][:, lo:b1],
                                      start=True, stop=True,
                                  )
                          et_t = et_pool.tile([JC, IH], bf16, tag="et",
                                              name=f"et{rep}_{p}_{half}_{c}")
                          nc.scalar.activation(et_t[:, x:IH], ps_t[:, x:IH],
                                               EXP)
                          if j0 >= i_lo:
                              # diagonal block: keep j <= i
                              nc.vector.tensor_mul(
                                  et_t[:, x:x + JC], et_t[:, x:x + JC],
                                  trim_t[:])
                          # O'^T += vp_chunk.T @ exp(S'^T); issue the bank
                          # that does not touch the trimmed block first
                          first = c == 0
                          for b0 in (512, 0):
                              lo = max(x, b0)
                              b1 = b0 + 512
                              if lo < b1:
                                  nc.tensor.matmul(
                                      po_t[:, lo:b1],
                                      vp_t[:, c * (D + 1):(c + 1) * (D + 1)],
                                      et_t[:, lo:b1],
                                      start=first,
                                      stop=(c == last_c[b0]),
                                  )
                      # po bank0 is final after the normal loop (its last
                      # accumulating chunk is nchunks-5): copy + store it now
                      # so the end-of-half chain only handles bank1
                      ot_t = ot_pool.tile([D + 1, IH], f32, tag="ot",
                                          name=f"ot{rep}_{p}_{half}")
                      nc.vector.tensor_copy(ot_t[:, 0:512], po_t[:, 0:512])
                      nc.sync.dma_start(out=o[p][:, i_lo:i_lo + 512],
                                        in_=ot_t[:, 0:512])
                      # Packed tail: the last 4 chunks of the half have causal
                      # widths 512,384,256,128 (all at x >= 512, all diagonal).
                      # Pack them contiguously into TWO ps tiles so 2 wide exps
                      # replace 4 narrow ones (each QK slice stays in one PSUM
                      # bank; each PV dst is within po bank1).
                      tail0 = nchunks - 4
                      for ti, group in ((0, ((0, 0), (1, 512))),
                                        (1, ((2, 0), (3, 256)))):
                          ps_t = ps_pool.tile([JC, IH], f32, tag="ps",
                                              name=f"ps{rep}_{p}_{half}_t{ti}")
                          for k, pos in group:
                              c = tail0 + k
                              j0 = JC * c
                              x = j0 - i_lo
                              w = IH - x
                              nc.tensor.matmul(
                                  ps_t[:, pos:pos + w],
                                  kt_t[:, j0:j0 + JC],
                                  qt_t[half][:, x:IH],
                                  start=True, stop=True,
                              )
                          width = group[1][1] + (IH - (JC * (tail0 + group[1][0]) - i_lo))
                          et_t = et_pool.tile([JC, IH], bf16, tag="et",
                                              name=f"et{rep}_{p}_{half}_t{ti}")
                          nc.scalar.activation(et_t[:, 0:width],
                                               ps_t[:, 0:width], EXP)
                          for k, pos in group:
                              c = tail0 + k
                              j0 = JC * c
                              x = j0 - i_lo
                              w = IH - x
                              nc.vector.tensor_mul(
                                  et_t[:, pos:pos + JC], et_t[:, pos:pos + JC],
                                  trim_t[:])
                              nc.tensor.matmul(
                                  po_t[:, x:IH],
                                  vp_t[:, c * (D + 1):(c + 1) * (D + 1)],
                                  et_t[:, pos:pos + w],
                                  start=False,
                                  stop=(c == last_c[512]),
                              )
                      nc.vector.tensor_copy(ot_t[:, 512:IH], po_t[:, 512:IH])
                      nc.sync.dma_start(out=o[p][:, i_lo + 512:i_lo + IH],
                                        in_=ot_t[:, 512:IH])

    nc.compile()
    _CACHED[key] = nc
    return nc


def _prep_core_inputs(queries, keys, values, tau, delta, core):
    qk = np.empty((PAIRS, E + 1, 2 * L), dtype=ml_dtypes.bfloat16)
    vp = np.empty((PAIRS, JC, N_CHUNKS * (D + 1)), dtype=ml_dtypes.bfloat16)
    for p in range(PAIRS):
        g = core * PAIRS + p
        b, h = g // H, g % H
        qk[p, :E, :L] = (SCALE * tau[b, 0]) * queries[b, :, h, :].T
        qk[p, E, :L] = 1.0
        qk[p, :E, L:] = keys[b, :, h, :].T
        qk[p, E, L:] = SCALE * delta[b, :]
        v = values[b, :, h, :].reshape(N_CHUNKS, JC, D)
        vch = vp[p].reshape(JC, N_CHUNKS, D + 1)
        vch[:, :, :D] = v.transpose(1, 0, 2)
        vch[:, :, D] = 1.0
    trim = np.triu(np.ones((JC, JC))).astype(ml_dtypes.bfloat16)
    return {"qk": qk, "vp": vp, "trim": trim}


def _run(queries, keys, values, tau, delta, trace=False, trace_kwargs=None):
    from concourse.bass_utils import run_bass_kernel_spmd

    queries = np.asarray(queries, dtype=np.float32)
    keys = np.asarray(keys, dtype=np.float32)
    values = np.asarray(values, dtype=np.float32)
    tau = np.asarray(tau, dtype=np.float32)
    delta = np.asarray(delta, dtype=np.float32)

    nc = _build_bass()
    in_maps = [
        _prep_core_inputs(queries, keys, values, tau, delta, core)
        for core in range(N_CORES)
    ]
    res = run_bass_kernel_spmd(
        nc, in_maps, list(range(N_CORES)), trace=trace,
        **(trace_kwargs or {}),
    )

    out = np.empty((B, L, H, D), dtype=np.float32)
    for core in range(N_CORES):
        o = res.results[core]["o"]  # [PAIRS, 65, L]
        for p in range(PAIRS):
            g = core * PAIRS + p
            b, h = g // H, g % H
            out[b, :, h, :] = (o[p, :D, :] / o[p, D:D + 1, :]).T
    return out, res


def kernel(queries, keys, values, tau, delta):
    out, _ = _run(queries, keys, values, tau, delta)
    return out
